# revision 1
# baseline (speedup 1.0000x reference)
"""BernNet (K=10) forward on 8 TRN2 NeuronCores.

Mathematical structure: the reference computes
    out = log_softmax( sum_i coef_i * relu(temp)_i * L^i (2I-L)^{K-i} h )
with h = relu(x@W1+b1)@W2+b2, L = I - A_hat, coef_i = C(K,i)/2^K.

Since L and 2I-L commute, sum_i C(K,i) relu(t)_i (I-A)^i (I+A)^{K-i} is a
degree-K polynomial in A with monomial coefficients c_j computable exactly
on the host.  With temp = ones (what reset_parameters produces, and what
setup_inputs supplies) the binomial theorem gives sum = (2I)^K / 2^K = I:
the propagation is the exact identity, so out = log_softmax(h).

The device kernel therefore evaluates the fused MLP + log_softmax,
node-sharded across the 8 cores (12800 padded nodes per core).  x is
uploaded pre-transposed ([512, nodes]) so features sit on SBUF partitions
and no on-device transpose of the activations is needed.  If temp were ever
not-identity (never happens for this problem's inputs), a host fallback
evaluates the polynomial exactly.
"""

import os
import numpy as np
from math import comb

K = 10
N_NODES = 100000
F = 512        # NUM_FEATURES
H = 256        # HIDDEN
C = 64         # NUM_CLASSES
NCORES = 8
CHUNK = 512                    # nodes per inner chunk (one PSUM bank)
SUPER = 5                      # chunks per output superchunk
NSH = 12544                    # padded nodes per core (24*512 + 256 = 98*128)
NPAD = NSH * NCORES            # 102400

# Set to "f32r" to use the fast replicated-fp32 tensor-engine mode.
MM_MODE = os.environ.get("KERNEL_MM_MODE", "f32r")

last_results = None            # BassKernelResults of the last device run


def _bern_poly_coeffs(temp):
    """Monomial coefficients c_j (in A) of sum_i coef_i*relu(temp_i)*(I-A)^i(I+A)^{K-i}.

    Exact: all intermediate values are integers * 2^-K, well under 2^53.
    """
    t = np.maximum(np.asarray(temp, dtype=np.float64), 0.0)
    c = np.zeros(K + 1)
    for i in range(K + 1):
        a = np.zeros(K + 1)
        for m in range(i + 1):
            for n in range(K - i + 1):
                a[m + n] += comb(i, m) * ((-1.0) ** m) * comb(K - i, n)
        c += (comb(K, i) / 2.0 ** K) * t[i] * a
    return c


def _build_nc(mm_mode, repeat=1):
    import concourse.bass as bass
    import concourse.mybir as mybir
    import concourse.tile as tile
    from concourse import bacc, masks
    from contextlib import ExitStack

    f32 = mybir.dt.float32
    # dtype of the matmul operand chain (DRAM + SBUF tiles feeding the PE)
    sdt = {"f32": f32, "f32r": mybir.dt.float32r,
           "bf16": mybir.dt.bfloat16, "f16": mybir.dt.float16}[mm_mode]
    AF = mybir.ActivationFunctionType

    nc = bacc.Bacc(None, target_bir_lowering=False)
    xTd = nc.dram_tensor("xT", (F, NSH), sdt, kind="ExternalInput")
    W1d = nc.dram_tensor("W1c", (128, 4, H), sdt, kind="ExternalInput")
    W2d = nc.dram_tensor("W2c", (128, 2, C), sdt, kind="ExternalInput")
    b1d = nc.dram_tensor("b1c", (128, 2), f32, kind="ExternalInput")
    b2d = nc.dram_tensor("b2c", (C, 1), f32, kind="ExternalInput")
    identd = nc.dram_tensor("ident64", (C, C), f32, kind="ExternalInput")
    # SBUF-mirrored layout [partition, row-block, class]; host unshuffles.
    # Keeps every output DMA descriptor a contiguous >=5KB run.
    outd = nc.dram_tensor("out", (128, NSH // 128, C), f32,
                          kind="ExternalOutput")

    with ExitStack() as ctx:
        tc = ctx.enter_context(tile.TileContext(nc))
        const = ctx.enter_context(tc.tile_pool(name="const", bufs=1))
        xpool = ctx.enter_context(tc.tile_pool(name="xt", bufs=8))
        obig = ctx.enter_context(tc.tile_pool(name="obig", bufs=2))
        h1pool = ctx.enter_context(tc.tile_pool(name="h1", bufs=4))
        h2pool = ctx.enter_context(tc.tile_pool(name="h2", bufs=3))
        opool = ctx.enter_context(tc.tile_pool(name="o", bufs=3))
        stat = ctx.enter_context(tc.tile_pool(name="stat", bufs=6))
        ps1p = ctx.enter_context(
            tc.tile_pool(name="ps1", bufs=4, space=bass.MemorySpace.PSUM))
        ps2p = ctx.enter_context(
            tc.tile_pool(name="ps2", bufs=2, space=bass.MemorySpace.PSUM))
        pstp = ctx.enter_context(
            tc.tile_pool(name="pst", bufs=2, space=bass.MemorySpace.PSUM))

        # The first layer-1 matmul needs only W1's k-chunk 0 (plus the
        # first 1/4 of x chunk 0, loaded in the main loop): load that
        # piece first, everything else after.
        W1sb = const.tile([128, 4, H], sdt)
        nc.sync.dma_start(W1sb[:, 0:1, :], W1d[:, 0:1, :])
        W2sb = const.tile([128, 2, C], sdt)
        b1sb = const.tile([128, 2], f32)
        b2sb = const.tile([C, 1], f32)
        ident = const.tile([C, C], f32)

        def load_rest_of_consts():
            nc.sync.dma_start(W1sb[:, 1:4, :], W1d[:, 1:4, :])
            nc.sync.dma_start(W2sb[:], W2d[:])
            nc.sync.dma_start(b1sb[:], b1d[:])
            nc.sync.dma_start(b2sb[:], b2d[:])
            nc.sync.dma_start(ident[:], identd[:])

        # Preload the one ACT table set holding Exp+Ln+Relu+Identity
        # (natural_log_exp_and_others).  Without this, the table-load
        # inserter greedily alternates exp_and_others <-> natural_log,
        # costing ~40 * 1.3us of ACT time.
        from concourse.hw_specs import get_activation_tables
        set_names = list(get_activation_tables(nc.m.arch).keys())
        nc.scalar.add_instruction(mybir.InstLoadActFuncSet(
            name=nc.get_next_instruction_name(),
            act_func_set_id=set_names.index("natural_log_exp_and_others"),
            ins=[], outs=[]))

        xTv = xTd.rearrange("(kc p) n -> p kc n", p=128)
        NB = CHUNK // 128

        # repeat>1 wraps the whole node loop in a hardware For_i so wall-
        # clock probes can measure per-iteration time; repeat==1 (the
        # production path) emits no loop at all.
        import contextlib
        loop_cm = (tc.For_i(0, repeat, 1,
                            hint_engines=(mybir.EngineType.PE,
                                          mybir.EngineType.Activation,
                                          mybir.EngineType.DVE,
                                          mybir.EngineType.SP))
                   if repeat > 1 else contextlib.nullcontext())

        def emit_mlp(xt, w):
            """Layers 1+2 for one w-node chunk -> h2 (w = 512 or 256)."""
            h1 = []
            for mh in range(2):
                ps1 = ps1p.tile([128, CHUNK], f32, tag="ps1")
                for kc in range(4):
                    nc.tensor.matmul(
                        ps1[:, :w],
                        W1sb[:, kc, mh * 128:(mh + 1) * 128],
                        xt[:, kc, :w],
                        start=(kc == 0),
                        stop=(kc == 3),
                    )
                h1t = h1pool.tile([128, CHUNK], sdt, tag="h1")
                nc.scalar.activation(h1t[:, :w], ps1[:, :w], AF.Relu,
                                     bias=b1sb[:, mh:mh + 1])
                h1.append(h1t)

            ps2 = ps2p.tile([C, CHUNK], f32, tag="ps2")
            for kh in range(2):
                nc.tensor.matmul(
                    ps2[:, :w],
                    W2sb[:, kh, :],
                    h1[kh][:, :w],
                    start=(kh == 0),
                    stop=(kh == 1),
                )
            # h2 = ps2 + b2 (per-partition bias) on DVE
            h2 = h2pool.tile([C, CHUNK], f32, tag="h2")
            nc.vector.tensor_scalar_add(h2[:, :w], ps2[:, :w], b2sb[:, 0:1])
            return h2

        def emit_tail(h2, outsb, boff, w):
            """Transpose + log_softmax for one chunk into outsb[boff:]."""
            nbt = w // 128
            # logits back to [node, class]: [128, nbt, C] in one bank
            pst = pstp.tile([128, NB, C], f32, tag="pst")
            for nb in range(nbt):
                nc.tensor.transpose(pst[:, nb, :], h2[:, bass.ts(nb, 128)],
                                    ident[:])
            # log_softmax over the class axis, whole chunk at once.
            # Logits are O(1) (weights are ~U(+-0.06), x ~ N(0,1)), so
            # exp() cannot overflow and the max-subtraction is skipped:
            # log_softmax(h) = h - log(sum(exp(h))) exactly.
            exps = opool.tile([128, NB, C], f32, tag="exps")
            nc.scalar.activation(exps[:, :nbt, :], pst[:, :nbt, :], AF.Exp)
            sums = stat.tile([128, NB], f32, tag="sums")
            nc.vector.reduce_sum(sums[:, :nbt], exps[:, :nbt, :],
                                 axis=mybir.AxisListType.X)
            logsum = stat.tile([128, NB], f32, tag="logsum")
            nc.scalar.activation(logsum[:, :nbt], sums[:, :nbt], AF.Ln)
            nc.vector.tensor_sub(outsb[:, boff:boff + nbt, :],
                                 pst[:, :nbt, :],
                                 logsum[:, :nbt].to_broadcast((128, nbt, C)))

        # Software pipeline: the tail of chunk t-1 is emitted between the
        # matmuls of chunk t so the PE never waits on the ACT/DVE chain.
        # x is loaded per 512-node chunk (deep ring buffer keeps the DMA
        # engines streaming); outputs are batched per SUPER chunks.
        with loop_cm:
            widths = [CHUNK] * 24 + [256]          # 24*512 + 256 = 12544
            n_chunks = len(widths)
            starts = [0]
            for w in widths:
                starts.append(starts[-1] + w)
            blk = [st // 128 for st in starts]     # global 128-row block offs
            pending = None     # (h2, outsb, boff_in_group, w, t, g)
            outsb = None
            for t in range(n_chunks):
                g, tt = divmod(t, SUPER)
                w = widths[t]
                xt = xpool.tile([128, 4, CHUNK], sdt, tag="xt")
                if t == 0:
                    # Split the very first load per k-chunk so the first
                    # matmul starts after ~1/4 of the transfer.
                    for kc in range(4):
                        nc.sync.dma_start(
                            xt[:, kc:kc + 1, :w],
                            xTv[:, kc:kc + 1, starts[t]:starts[t] + w])
                    load_rest_of_consts()
                else:
                    nc.sync.dma_start(xt[:, :, :w],
                                      xTv[:, :, starts[t]:starts[t] + w])
                if tt == 0:
                    outsb = obig.tile([128, SUPER * NB, C], f32, tag="outsb")

                h2 = emit_mlp(xt, w)
                if pending is not None:
                    ph2, poutsb, pboff, pw, pt, pg = pending
                    emit_tail(ph2, poutsb, pboff, pw)
                    if pt % SUPER == SUPER - 1:
                        # finished group pg: flush its output block range
                        g0 = blk[pg * SUPER]
                        nb_g = blk[pt] + pw // 128 - g0
                        nc.sync.dma_start(outd[:, g0:g0 + nb_g, :],
                                          poutsb[:, :nb_g, :])
                pending = (h2, outsb, blk[t] - blk[g * SUPER], w, t, g)

            # final chunk's tail, then per-chunk stores for the last group
            ph2, poutsb, pboff, pw, pt, pg = pending
            emit_tail(ph2, poutsb, pboff, pw)
            g0 = blk[pg * SUPER]
            for t2 in range(pg * SUPER, n_chunks):
                nbt = widths[t2] // 128
                nc.sync.dma_start(
                    outd[:, blk[t2]:blk[t2] + nbt, :],
                    poutsb[:, blk[t2] - g0:blk[t2] - g0 + nbt, :])

    nc.compile()
    return nc


_nc_cache = {}


def _get_nc(mm_mode):
    if mm_mode not in _nc_cache:
        _nc_cache[mm_mode] = _build_nc(mm_mode)
    return _nc_cache[mm_mode]


def _run_device_mlp(x, W1, b1, W2, b2, mm_mode=None, trace=False):
    """log_softmax(relu(x@W1+b1)@W2+b2) on the 8 cores; returns [N_NODES, C]."""
    from concourse import bass_utils
    global last_results

    if mm_mode is None:
        mm_mode = MM_MODE
    nc = _get_nc(mm_mode)

    sdt_np = np.float32
    if mm_mode == "bf16":
        import ml_dtypes
        sdt_np = ml_dtypes.bfloat16
    elif mm_mode == "f16":
        sdt_np = np.float16

    x = np.asarray(x, dtype=np.float32)
    W1c = np.ascontiguousarray(
        np.asarray(W1, np.float32).reshape(4, 128, H).transpose(1, 0, 2)
    ).astype(sdt_np)
    W2c = np.ascontiguousarray(
        np.asarray(W2, np.float32).reshape(2, 128, C).transpose(1, 0, 2)
    ).astype(sdt_np)
    b1c = np.ascontiguousarray(np.asarray(b1, np.float32).reshape(2, 128).T)
    b2c = np.ascontiguousarray(np.asarray(b2, np.float32).reshape(C, 1))
    ident64 = np.eye(C, dtype=np.float32)

    in_maps = []
    for c in range(NCORES):
        lo = c * NSH
        hi = min((c + 1) * NSH, N_NODES)
        if hi - lo == NSH:
            xTc = np.ascontiguousarray(x[lo:hi].T.astype(sdt_np, copy=False))
        else:
            xTc = np.zeros((F, NSH), dtype=sdt_np)
            if hi > lo:
                xTc[:, :hi - lo] = x[lo:hi].T
        in_maps.append({
            "xT": xTc, "W1c": W1c, "W2c": W2c, "b1c": b1c, "b2c": b2c,
            "ident64": ident64,
        })

    res = None
    for attempt in range(3):
        try:
            res = bass_utils.run_bass_kernel_spmd(
                nc, in_maps, core_ids=list(range(NCORES)),
                trace=trace and attempt == 0)
            break
        except ModuleNotFoundError:
            # NTFF profiling hook unavailable in this container; retry
            # untraced.
            trace = False
        except Exception:
            # Transient runtime failure: retry once more, then give up so
            # the caller can fall back to the host path.
            if attempt == 2:
                raise
    last_results = res
    out = np.concatenate([
        res.results[c]["out"].transpose(1, 0, 2).reshape(NSH, C)
        for c in range(NCORES)
    ], axis=0)
    return np.ascontiguousarray(out[:N_NODES])


def _host_reference_fallback(x, edge_index, W1, b1, W2, b2, temp):
    """Exact host evaluation for general temp (never hit for this problem)."""
    import scipy.sparse as sp

    x = np.asarray(x, np.float32)
    h = np.maximum(x @ np.asarray(W1, np.float32) + np.asarray(b1, np.float32), 0)
    h = h @ np.asarray(W2, np.float32) + np.asarray(b2, np.float32)

    src = np.asarray(edge_index[0]).astype(np.int64)
    dst = np.asarray(edge_index[1]).astype(np.int64)
    deg = np.bincount(src, minlength=N_NODES).astype(np.float32)
    dis = np.where(deg > 0, 1.0 / np.sqrt(np.maximum(deg, 1e-30)), 0.0)
    w = (dis[src] * dis[dst]).astype(np.float32)
    A = sp.csr_matrix((w, (dst, src)), shape=(N_NODES, N_NODES), dtype=np.float32)

    TEMP = np.maximum(np.asarray(temp, np.float32), 0.0)
    coef = np.array([comb(K, i) / 2.0 ** K for i in range(K + 1)], np.float32)

    tmp = [h]
    for _ in range(K):
        h = h + A @ h
        tmp.append(h)
    out = coef[0] * TEMP[0] * tmp[K]
    for i in range(K):
        y = tmp[K - i - 1]
        for _ in range(i + 1):
            y = y - A @ y
        out = out + coef[i + 1] * TEMP[i + 1] * y

    m = out.max(axis=1, keepdims=True)
    e = np.exp(out - m)
    return (out - m - np.log(e.sum(axis=1, keepdims=True))).astype(np.float32)


def kernel(x, edge_index, W1, b1, W2, b2, temp, **_unused):
    c = _bern_poly_coeffs(temp)
    is_identity = abs(c[0] - 1.0) < 1e-9 and np.all(np.abs(c[1:]) < 1e-9)
    if not is_identity:
        return _host_reference_fallback(x, edge_index, W1, b1, W2, b2, temp)
    return _run_device_mlp(x, W1, b1, W2, b2)



# revision 51
# speedup vs baseline: 1.8218x; 1.8218x over previous
"""BernNet (K=10) forward on 8 TRN2 NeuronCores.

Mathematical structure: the reference computes
    out = log_softmax( sum_i coef_i * relu(temp)_i * L^i (2I-L)^{K-i} h )
with h = relu(x@W1+b1)@W2+b2, L = I - A_hat, coef_i = C(K,i)/2^K.

Since L and 2I-L commute, sum_i C(K,i) relu(t)_i (I-A)^i (I+A)^{K-i} is a
degree-K polynomial in A with monomial coefficients computable exactly on
the host.  With temp = ones (what reset_parameters produces, and what
setup_inputs supplies) the binomial theorem gives sum = (2I)^K / 2^K = I:
the propagation is the exact identity, so out = log_softmax(h).

The device kernel evaluates the fused MLP + log_softmax, node-sharded
across the 8 cores (12800 padded nodes per core, 25 chunks of 512).

Layer 1 (512->256) runs on the PE in fp8-e4m3 with DoubleRow perf mode
(K=256 per instruction, 2x ALU throughput).  x and W1 are scaled by 16 on
the host to keep fp8 values out of the subnormal range; the exact 1/256
unscale is folded into the relu activation's scale operand.  Layer 2
(256->64) runs in bf16 with node-blocks as the stationary operand so the
logits land in [node, class] orientation and no PE transpose is needed.
b2 is accumulated into the same PSUM tile by a ones(1/128)-matmul.  The
relu is split across ACT and DVE (the DVE path uses relu's positive
homogeneity: it skips the 1/256 unscale and its node-blocks use a
pre-divided W2 copy in layer 2).  log_softmax reduces over the free axis
(classes): ACT exp -> DVE reduce -> ACT ln -> DVE broadcast-subtract,
batched over chunk pairs.  Output is written f32, [128, row-block, class];
the host unshuffles.
"""

import os
import numpy as np
from math import comb

K = 10
N_NODES = 100000
F = 512        # NUM_FEATURES
H = 256        # HIDDEN
C = 64         # NUM_CLASSES
NCORES = 8
CHUNK = 512                    # nodes per chunk (one PSUM bank of f32)
NCH = 25                       # chunks per core
NSH = NCH * CHUNK              # 12800 padded nodes per core
NPAIRS = NCH // 2              # 12 full pairs; chunk 24 is a solo tail
GRP = int(os.environ.get("KERNEL_GRP", "2"))   # pairs per output-store group
XBUFS = int(os.environ.get("KERNEL_XBUFS", "8"))
H1BUFS = int(os.environ.get("KERNEL_H1BUFS", "3"))
EBUFS = int(os.environ.get("KERNEL_EBUFS", "2"))
# relu: ACT does mh1 nodes [0,SPLIT), DVE the rest (multiple of 128).
# SPLIT=0 measures fastest: mh1 is then written by ONE engine (DVE), so
# layer-2 matmuls never wait on a late-scheduled small ACT instruction.
SPLIT = int(os.environ.get("KERNEL_SPLIT", "0"))
FP8_SCALE = 16.0               # host-side scale on x and W1 in fp8 mode

MM_MODE = os.environ.get("KERNEL_MM_MODE", "fp8")

last_results = None            # BassKernelResults of the last device run


def _bern_poly_coeffs(temp):
    """Monomial coefficients c_j (in A) of sum_i coef_i*relu(temp_i)*(I-A)^i(I+A)^{K-i}.

    Exact: all intermediate values are integers * 2^-K, well under 2^53.
    """
    t = np.maximum(np.asarray(temp, dtype=np.float64), 0.0)
    c = np.zeros(K + 1)
    for i in range(K + 1):
        a = np.zeros(K + 1)
        for m in range(i + 1):
            for n in range(K - i + 1):
                a[m + n] += comb(i, m) * ((-1.0) ** m) * comb(K - i, n)
        c += (comb(K, i) / 2.0 ** K) * t[i] * a
    return c


def _build_nc(mm_mode, repeat=1):
    import concourse.bass as bass
    import concourse.mybir as mybir
    import concourse.tile as tile
    from concourse import bacc
    from contextlib import ExitStack
    import contextlib

    f32 = mybir.dt.float32
    bf16 = mybir.dt.bfloat16
    is_fp8 = mm_mode.startswith("fp8")
    use_dr = mm_mode == "fp8"
    use_swi = mm_mode == "fp8swi"
    l1pair = os.environ.get("KERNEL_L1PAIR", "0") == "1"
    # timing-only ablations (break numerics): csv of
    # xdma,out,l1,l2,smax
    ablate = set(os.environ.get("KERNEL_ABLATE", "").split(",")) - {""}
    out_bf16 = os.environ.get("KERNEL_OUT", "f32") == "bf16"
    odt = bf16 if out_bf16 else f32
    sdt = mybir.dt.float8e4 if is_fp8 else bf16
    sc = 1.0 / (FP8_SCALE * FP8_SCALE) if is_fp8 else 1.0
    AF = mybir.ActivationFunctionType
    ALU = mybir.AluOpType
    DR = mybir.MatmulPerfMode.DoubleRow
    DRS = mybir.MatmulPerfMode.DoubleRowSwInterleave

    nc = bacc.Bacc(None, target_bir_lowering=False)
    xTd = nc.dram_tensor("xT", (128, NCH, 4, CHUNK), sdt, kind="ExternalInput")
    W1d = nc.dram_tensor("W1c", (128, 4, H), sdt, kind="ExternalInput")
    W2d = nc.dram_tensor("W2c", (128, 2, 2, C), bf16, kind="ExternalInput")
    b1d = nc.dram_tensor("b1c", (128, 3), f32, kind="ExternalInput")
    b2d = nc.dram_tensor("b2q", (128, 4 * C), bf16, kind="ExternalInput")
    invd = nc.dram_tensor("inv128", (128, 128), bf16, kind="ExternalInput")
    # SBUF-mirrored layout [partition, row-block, class]; host unshuffles.
    outd = nc.dram_tensor("out", (128, NSH // 128, C), odt,
                          kind="ExternalOutput")

    with ExitStack() as ctx:
        tc = ctx.enter_context(tile.TileContext(nc))
        const = ctx.enter_context(tc.tile_pool(name="const", bufs=1))
        xpool = ctx.enter_context(tc.tile_pool(name="xt", bufs=XBUFS))
        h1p = ctx.enter_context(tc.tile_pool(name="h1", bufs=H1BUFS))
        expp = ctx.enter_context(tc.tile_pool(name="exps", bufs=EBUFS))
        sump = ctx.enter_context(tc.tile_pool(name="sums", bufs=EBUFS))
        lsp = ctx.enter_context(tc.tile_pool(name="ls", bufs=EBUFS))
        obig = ctx.enter_context(tc.tile_pool(name="obig", bufs=2))
        ps1p = ctx.enter_context(
            tc.tile_pool(name="ps1", bufs=2, space=bass.MemorySpace.PSUM))
        pstp = ctx.enter_context(
            tc.tile_pool(name="pst", bufs=3, space=bass.MemorySpace.PSUM))

        W1sb = const.tile([128, 4, H], sdt)
        W2sb = const.tile([128, 2, 2, C], bf16)
        b1sb = const.tile([128, 3], f32)
        b2qsb = const.tile([128, 4 * C], bf16)
        invsb = const.tile([128, 128], bf16)
        nc.sync.dma_start(W1sb[:], W1d[:])
        nc.sync.dma_start(W2sb[:], W2d[:])
        nc.sync.dma_start(b1sb[:], b1d[:])
        nc.sync.dma_start(b2qsb[:], b2d[:])
        nc.sync.dma_start(invsb[:], invd[:])

        # Preload the one ACT table set holding Exp+Ln+Relu (the table-load
        # inserter otherwise alternates table sets at ~1.3us per switch).
        from concourse.hw_specs import get_activation_tables
        set_names = list(get_activation_tables(nc.m.arch).keys())
        nc.scalar.add_instruction(mybir.InstLoadActFuncSet(
            name=nc.get_next_instruction_name(),
            act_func_set_id=set_names.index("natural_log_exp_and_others"),
            ins=[], outs=[]))

        loop_cm = (tc.For_i(0, repeat, 1,
                            hint_engines=(mybir.EngineType.PE,
                                          mybir.EngineType.Activation,
                                          mybir.EngineType.DVE,
                                          mybir.EngineType.SP))
                   if repeat > 1 else contextlib.nullcontext())

        def emit_l1_mms(ps1, xt, p4, mh):
            if use_swi:
                # weights pre-interleaved+column-reversed on the host in
                # W1sb[:, 2*p4+mh, :]; contiguous 256-col weight load
                nc.tensor.matmul(
                    ps1[:, mh, :],
                    W1sb[:, 2 * p4 + mh, :],
                    xt[:, 0, 2 * p4:2 * p4 + 2, :],
                    start=(p4 == 0), stop=(p4 == 1),
                    perf_mode=DRS)
            elif use_dr:
                nc.tensor.matmul(
                    ps1[:, mh, :],
                    W1sb[:, 2 * p4:2 * p4 + 2, bass.ts(mh, 128)],
                    xt[:, 0, 2 * p4:2 * p4 + 2, :],
                    start=(p4 == 0), stop=(p4 == 1),
                    perf_mode=DR)
            else:
                for i2 in range(2):
                    kc = 2 * p4 + i2
                    nc.tensor.matmul(
                        ps1[:, mh, :],
                        W1sb[:, kc, bass.ts(mh, 128)],
                        xt[:, 0, kc, :],
                        start=(kc == 0), stop=(kc == 3))

        def emit_relu(ps1):
            h1 = h1p.tile([128, 2, CHUNK], bf16, tag="h1")
            nc.scalar.activation(h1[:, 0, :], ps1[:, 0, :], AF.Relu,
                                 bias=b1sb[:, 0:1], scale=sc)
            if SPLIT > 0:
                nc.scalar.activation(h1[:, 1, :SPLIT], ps1[:, 1, :SPLIT],
                                     AF.Relu, bias=b1sb[:, 1:2], scale=sc)
            # DVE relu of the remaining mh1 nodes: relu is positively
            # homogeneous, so skip the 1/256 unscale here (bias pre-scaled
            # by 256); these node-blocks use W2/256 in layer 2.
            if SPLIT < CHUNK:
                nc.vector.tensor_scalar(h1[:, 1, SPLIT:], ps1[:, 1, SPLIT:],
                                        b1sb[:, 2:3], 0.0, ALU.add, ALU.max)
            return h1

        xt_shared = [None]

        def emit_front(t):
            """DMA + layer 1 + relu for chunk t; returns the h1 tile."""
            if "xdma" in ablate:
                if xt_shared[0] is None:
                    xt0 = const.tile([128, 1, 4, CHUNK], sdt)
                    nc.sync.dma_start(xt0[:], xTd[:, 0:1, :, :])
                    xt_shared[0] = xt0
                xt = xt_shared[0]
            else:
                xt = xpool.tile([128, 1, 4, CHUNK], sdt, tag="xt")
                nc.sync.dma_start(xt[:], xTd[:, t:t + 1, :, :])
            ps1 = ps1p.tile([128, 2, CHUNK], f32, tag="ps1")
            if "l1" in ablate:
                for mh in range(2):
                    if use_dr:
                        nc.tensor.matmul(
                            ps1[:, mh, :], W1sb[:, 0:2, bass.ts(mh, 128)],
                            xt[:, 0, 0:2, :], start=True, stop=True,
                            perf_mode=DR)
                    else:
                        nc.tensor.matmul(
                            ps1[:, mh, :], W1sb[:, 0, bass.ts(mh, 128)],
                            xt[:, 0, 0, :], start=True, stop=True)
            else:
                for p4 in range(2):
                    for mh in range(2):
                        emit_l1_mms(ps1, xt, p4, mh)
            return emit_relu(ps1)

        def emit_front_pair(ta, tb):
            """Both chunks' L1 interleaved so consecutive matmuls share the
            same stationary W1 slice (amortizes LDWEIGHTS)."""
            xta = xpool.tile([128, 1, 4, CHUNK], sdt, tag="xt")
            nc.sync.dma_start(xta[:], xTd[:, ta:ta + 1, :, :])
            xtb = xpool.tile([128, 1, 4, CHUNK], sdt, tag="xt")
            nc.sync.dma_start(xtb[:], xTd[:, tb:tb + 1, :, :])
            ps1a = ps1p.tile([128, 2, CHUNK], f32, tag="ps1")
            ps1b = ps1p.tile([128, 2, CHUNK], f32, tag="ps1")
            for p4 in range(2):
                for mh in range(2):
                    emit_l1_mms(ps1a, xta, p4, mh)
                    emit_l1_mms(ps1b, xtb, p4, mh)
            return emit_relu(ps1a), emit_relu(ps1b)

        def emit_l2(h1, pst, poff):
            """Layer 2 for one chunk into pst[:, poff:poff+4, :]."""
            # b2, broadcast down 128 partitions via (1/128)-ones matmul;
            # start=True clears the accumulation region first.
            nc.tensor.matmul(pst[:, poff:poff + 4, :], invsb[:], b2qsb[:],
                             start=True, stop="l2" in ablate,
                             skip_group_check=True)
            if "l2" in ablate:
                return
            for nb in range(4):
                for kh in range(2):
                    sm = 1 if (kh == 1 and nb * 128 >= SPLIT) else 0
                    nc.tensor.matmul(
                        pst[:, poff + nb, :],
                        h1[:, kh, bass.ts(nb, 128)],
                        W2sb[:, kh, sm, :],
                        start=False, stop=(kh == 1), skip_group_check=True)

        def emit_exp(pst, nblk):
            if "smax" in ablate:
                return pst
            exps = expp.tile([128, 8, C], bf16, tag="exps")
            nc.scalar.activation(exps[:, :nblk, :], pst[:, :nblk, :], AF.Exp)
            return exps

        sumdt = bf16 if os.environ.get("KERNEL_SUMDT", "f32") == "bf16" else f32

        def emit_reduce(exps, nblk):
            sums = sump.tile([128, 8], sumdt, tag="sums")
            if sumdt is bf16:
                with nc.allow_low_precision(reason="bf16 sum of 64 exps"):
                    nc.vector.reduce_sum(sums[:, :nblk], exps[:, :nblk, :],
                                         axis=mybir.AxisListType.X)
            else:
                nc.vector.reduce_sum(sums[:, :nblk], exps[:, :nblk, :],
                                     axis=mybir.AxisListType.X)
            return sums

        def emit_ln(sums, nblk):
            if "smax" in ablate:
                return sums
            ls = lsp.tile([128, 8], f32, tag="ls")
            nc.scalar.activation(ls[:, :nblk], sums[:, :nblk], AF.Ln)
            return ls

        def emit_sub(pst, ls, outsb, boff, nblk):
            nc.vector.tensor_sub(outsb[:, boff:boff + nblk, :],
                                 pst[:, :nblk, :],
                                 ls[:, :nblk].to_broadcast((128, nblk, C)))

        def emit_store(dst, src):
            if "out" not in ablate:
                nc.sync.dma_start(dst, src)

        with loop_cm:
            # The softmax chain (exp -> reduce -> ln -> sub) is a 4-hop
            # cross-engine dependency; giving it TWO pair-periods to drain
            # (tails lag by 2 pairs, pst bufs=3) keeps its latency off the
            # PE/ACT critical path.
            state = {"outsb": None}

            def emit_tail_chain(tail, frame):
                ppst, pk = tail
                if pk % GRP == 0:
                    state["outsb"] = obig.tile([128, GRP * 8, C], odt,
                                               name="outsb", tag="outsb")
                exps = emit_exp(ppst, 8)
                if frame:
                    frame.pop(0)()
                sums = emit_reduce(exps, 8)
                while frame:
                    frame.pop(0)()
                ls = emit_ln(sums, 8)
                emit_sub(ppst, ls, state["outsb"], (pk % GRP) * 8, 8)
                if pk % GRP == GRP - 1:
                    g0 = (pk - GRP + 1) * 8
                    emit_store(outd[:, g0:g0 + GRP * 8, :],
                               state["outsb"][:])

            pending = []       # [(pst, pair_idx)]
            for k in range(NPAIRS):
                pst = pstp.tile([128, 8, C], f32, tag="pst")
                if l1pair:
                    h1a, h1b = emit_front_pair(2 * k, 2 * k + 1)
                else:
                    h1a = emit_front(2 * k)
                    h1b = emit_front(2 * k + 1)
                frame = [lambda: emit_l2(h1a, pst, 0),
                         lambda: emit_l2(h1b, pst, 4)]
                if len(pending) == 2:
                    emit_tail_chain(pending.pop(0), frame)
                else:
                    for f in frame:
                        f()
                pending.append((pst, k))

            # tail chunk 24 (solo) + the two pending pair tails
            pstT = pstp.tile([128, 8, C], f32, tag="pst")
            h1t = emit_front(NCH - 1)
            emit_tail_chain(pending.pop(0), [lambda: emit_l2(h1t, pstT, 0)])
            emit_tail_chain(pending.pop(0), [])
            outsbT = obig.tile([128, GRP * 8, C], odt, tag="outsb")
            expsT = emit_exp(pstT, 4)
            sumsT = emit_reduce(expsT, 4)
            lsT = emit_ln(sumsT, 4)
            emit_sub(pstT, lsT, outsbT, 0, 4)
            emit_store(outd[:, (NCH - 1) * 4:, :], outsbT[:, :4, :])

    nc.compile()
    return nc


_nc_cache = {}


def _get_nc(mm_mode, repeat=1):
    key = (mm_mode, repeat, os.environ.get("KERNEL_L1PAIR", "0"), SPLIT,
           os.environ.get("KERNEL_ABLATE", ""),
           os.environ.get("KERNEL_OUT", "f32"),
           os.environ.get("KERNEL_SUMDT", "f32"),
           GRP, XBUFS, H1BUFS, EBUFS)
    if key not in _nc_cache:
        _nc_cache[key] = _build_nc(mm_mode, repeat=repeat)
    return _nc_cache[key]


def _prep_in_maps(x, W1, b1, W2, b2, mm_mode):
    """Per-core input dicts: shard/pad/transpose/quantize on the host."""
    import ml_dtypes
    bf = ml_dtypes.bfloat16
    if mm_mode == "fp8":
        sdt_np = ml_dtypes.float8_e4m3
        s = FP8_SCALE
    else:
        sdt_np = bf
        s = 1.0
    sc = s * s

    W1s = np.asarray(W1, np.float32) * s
    if mm_mode == "fp8swi":
        # DoubleRowSwInterleave weight layout: per (p4, mh) block the 256
        # columns are (A/B-interleaved, column-reversed): see bass_interp
        # InstMatmult DoubleRowSwInterleave.  W1c[p, 2*p4+mh, 2j+i] =
        # W1s[(2*p4+i)*128+p, mh*128 + 127-j].
        T = W1s.reshape(2, 2, 128, 2, 128).transpose(2, 0, 3, 4, 1)
        W1c = np.ascontiguousarray(
            T[:, :, :, ::-1, :].reshape(128, 4, 2 * 128)).astype(sdt_np)
    else:
        W1c = np.ascontiguousarray(
            W1s.reshape(4, 128, H).transpose(1, 0, 2)).astype(sdt_np)
    w2b = np.asarray(W2, np.float32).reshape(2, 128, C).transpose(1, 0, 2)
    W2c = np.ascontiguousarray(
        np.stack([w2b, w2b / sc], axis=2)).astype(bf)          # [128,2,2,C]
    b1r = np.asarray(b1, np.float32).reshape(2, 128).T          # [128,2]
    b1c = np.ascontiguousarray(
        np.concatenate([b1r, b1r[:, 1:2] * sc], axis=1)).astype(np.float32)
    b2qc = np.ascontiguousarray(np.broadcast_to(
        np.tile(np.asarray(b2, np.float32), 4), (128, 4 * C))).astype(bf)
    invc = np.full((128, 128), 1.0 / 128.0, dtype=bf)

    xf = np.asarray(x, np.float32)
    in_maps = []
    for c in range(NCORES):
        lo = c * NSH
        hi = min(lo + NSH, N_NODES)
        xp = np.zeros((NSH, F), np.float32)
        xp[:hi - lo] = xf[lo:hi]
        if s != 1.0:
            xp *= s
        # [128 part, chunk, kc, node-in-chunk]; 2KB contiguous per
        # (partition, chunk) in fp8 -> efficient DMA descriptors.
        xTc = np.ascontiguousarray(
            xp.reshape(NCH, CHUNK, 4, 128).transpose(3, 0, 2, 1)
        ).astype(sdt_np)
        in_maps.append({
            "xT": xTc, "W1c": W1c, "W2c": W2c, "b1c": b1c, "b2q": b2qc,
            "inv128": invc,
        })
    return in_maps


def _gather(res):
    out = np.concatenate([
        res.results[c]["out"].astype(np.float32).transpose(1, 0, 2)
        .reshape(NSH, C)
        for c in range(NCORES)
    ], axis=0)
    return np.ascontiguousarray(out[:N_NODES])


def _run_device_mlp(x, W1, b1, W2, b2, mm_mode=None, trace=False):
    """log_softmax(relu(x@W1+b1)@W2+b2) on the 8 cores; returns [N_NODES, C]."""
    from concourse import bass_utils
    global last_results

    if mm_mode is None:
        mm_mode = MM_MODE
    nc = _get_nc(mm_mode)
    in_maps = _prep_in_maps(x, W1, b1, W2, b2, mm_mode)

    res = None
    for attempt in range(3):
        try:
            res = bass_utils.run_bass_kernel_spmd(
                nc, in_maps, core_ids=list(range(NCORES)),
                trace=trace and attempt == 0)
            break
        except ModuleNotFoundError:
            # NTFF profiling hook unavailable in this container; retry
            # untraced.
            trace = False
        except Exception:
            if attempt == 2:
                raise
    last_results = res
    return _gather(res)


def _host_reference_fallback(x, edge_index, W1, b1, W2, b2, temp):
    """Exact host evaluation for general temp (never hit for this problem)."""
    import scipy.sparse as sp

    x = np.asarray(x, np.float32)
    h = np.maximum(x @ np.asarray(W1, np.float32) + np.asarray(b1, np.float32), 0)
    h = h @ np.asarray(W2, np.float32) + np.asarray(b2, np.float32)

    src = np.asarray(edge_index[0]).astype(np.int64)
    dst = np.asarray(edge_index[1]).astype(np.int64)
    deg = np.bincount(src, minlength=N_NODES).astype(np.float32)
    dis = np.where(deg > 0, 1.0 / np.sqrt(np.maximum(deg, 1e-30)), 0.0)
    w = (dis[src] * dis[dst]).astype(np.float32)
    A = sp.csr_matrix((w, (dst, src)), shape=(N_NODES, N_NODES), dtype=np.float32)

    TEMP = np.maximum(np.asarray(temp, np.float32), 0.0)
    coef = np.array([comb(K, i) / 2.0 ** K for i in range(K + 1)], np.float32)

    tmp = [h]
    for _ in range(K):
        h = h + A @ h
        tmp.append(h)
    out = coef[0] * TEMP[0] * tmp[K]
    for i in range(K):
        y = tmp[K - i - 1]
        for _ in range(i + 1):
            y = y - A @ y
        out = out + coef[i + 1] * TEMP[i + 1] * y

    m = out.max(axis=1, keepdims=True)
    e = np.exp(out - m)
    return (out - m - np.log(e.sum(axis=1, keepdims=True))).astype(np.float32)


def kernel(x, edge_index, W1, b1, W2, b2, temp, **_unused):
    c = _bern_poly_coeffs(temp)
    is_identity = abs(c[0] - 1.0) < 1e-9 and np.all(np.abs(c[1:]) < 1e-9)
    if not is_identity:
        return _host_reference_fallback(x, edge_index, W1, b1, W2, b2, temp)
    return _run_device_mlp(x, W1, b1, W2, b2)
